# revision 1
# baseline (speedup 1.0000x reference)
"""Trainium2 Bass kernel for the MeshCNN-style GNN message-passing block.

Math: the reference is
    out[b,:,e] = Wa_fuse @ [conv1(x0); mesh_conv(x0)] + Wb_fuse @ [conv1(x1); mesh_conv(x1)] + biases
Everything after the neighbor gather / abs is linear, so the whole block
collapses to ten [3,128] effective matrices applied to:
    x (direct), f1+f3, f2+f4, |f1-f3|, |f2-f4|      (for x0 and x1)
plus one bias 3-vector.  The kernel is gather-dominated (memory regime).

Strategy (8 cores, SPMD):
- fp16 gather tables tab[b] = [x0[b];x1[b]] per-edge rows [E, 256] (512B rows),
  split into lo/hi halves of 30000 rows so indices fit dma_gather's int16.
- dma_gather(transpose=True) lands neighbor features channel-major [128,2,N],
  exactly what the PE matmul wants; |f1-f3| via DVE subtract + abs_max.
- Edges are dealt to cores grouped by which half their (swap-normalized)
  neighbor pair hits (LL/LH/HH), so each gather instruction reads one table
  half.  Pair (f1,f3) and pair (f2,f4) use independent permutations (two
  passes); the host sums the per-pass partial outputs.
"""

import hashlib
import os
import shutil

import numpy as np

import concourse.bass as bass
import concourse.bacc as bacc
import concourse.tile as tile
from concourse import mybir
from concourse.bass_utils import run_bass_kernel_spmd

# ---- NEFF compile cache: neuronxcc takes ~40 min for this program; cache the
# compiled NEFF keyed on exact BIR bytes so repeat invocations are fast. ----
_NEFF_CACHE = os.environ.get("KERNEL_NEFF_CACHE", "/tmp/neff_cache")
try:
    import concourse.bass2jax as _b2j

    if not hasattr(_b2j, "_orig_compile_bir_kernel"):
        _b2j._orig_compile_bir_kernel = _b2j.compile_bir_kernel

        def _cached_compile_bir_kernel(bir_json, tmpdir, neff_name="file.neff"):
            os.makedirs(_NEFF_CACHE, exist_ok=True)
            key = hashlib.sha256(bir_json).hexdigest()
            cpath = os.path.join(_NEFF_CACHE, key + ".neff")
            out = os.path.join(tmpdir, neff_name)
            if os.path.exists(cpath):
                shutil.copyfile(cpath, out)
                return out
            path = _b2j._orig_compile_bir_kernel(bir_json, tmpdir, neff_name)
            tmp = cpath + ".tmp"
            shutil.copyfile(path, tmp)
            os.replace(tmp, cpath)
            return path

        _b2j.compile_bir_kernel = _cached_compile_bir_kernel

    _b2j.install_neuronx_cc_hook()
    import libneuronxla as _lnx

    if hasattr(_lnx, "orig_neuronx_cc") and not hasattr(_lnx, "_ant_cc_cached"):
        _lnx._ant_cc_cached = True
        _orig_cc = _lnx.orig_neuronx_cc

        def _cached_cc(code, code_format, platform_version, file_prefix):
            os.makedirs(_NEFF_CACHE, exist_ok=True)
            key = hashlib.sha256(
                bytes(code) + bytes(code_format) + str(platform_version).encode()
            ).hexdigest()
            cpath = os.path.join(_NEFF_CACHE, key + ".cc")
            if os.path.exists(cpath):
                with open(cpath, "rb") as f:
                    return 0, f.read()
            r = _orig_cc(code, code_format, platform_version, file_prefix)
            try:
                rc, blob = r
                if rc == 0 and isinstance(blob, (bytes, bytearray)):
                    tmp = cpath + ".tmp"
                    with open(tmp, "wb") as f:
                        f.write(blob)
                    os.replace(tmp, cpath)
            except Exception:
                pass
            return r

        _lnx.orig_neuronx_cc = _cached_cc
except Exception:
    pass

B, C, E = 2, 128, 60000
HALF = 30000
NCORES = 8
EPC = E // NCORES              # 7500 direct edges per core
DPAD = 7680                    # direct width padded to 15*512
CAPS = (2048, 3968, 2048)      # per-core caps for classes LL, LH, HH
SEG_OFF = (0, 2048, 6016)
NPASS = sum(CAPS)              # 8064
NIDX16 = NPASS // 16           # 504 wrapped-idx columns per (b,pass,slot)
# table half used by (segment, slot): LL=(lo,lo) LH=(lo,hi) HH=(hi,hi)
SEG_HALVES = ((0, 0), (0, 1), (1, 1))

F16 = mybir.dt.float16
F32 = mybir.dt.float32
I16 = mybir.dt.int16

_compiled = None


def _mm_slices(n):
    """Split [0,n) into PSUM-bank-aligned matmul slices of <=512."""
    out = []
    a = 0
    while a < n:
        w = min(512, n - a)
        out.append((a, w))
        a += w
    return out


def _build_program():
    nc = bacc.Bacc("TRN2", target_bir_lowering=False, debug=False,
                   num_devices=NCORES)

    tabs = {}
    for b in range(B):
        for h in range(2):
            tabs[(b, h)] = nc.dram_tensor(f"tab{b}{h}", [HALF, 256], F16,
                                          kind="ExternalInput")
    xcm_d = nc.dram_tensor("xcm", [B, 128, 2, DPAD], F16, kind="ExternalInput")
    idx_d = nc.dram_tensor("idxs", [128, 8 * NIDX16], I16, kind="ExternalInput")
    wts_d = nc.dram_tensor("wts", [128, 30], F16, kind="ExternalInput")
    bias_d = nc.dram_tensor("bias3", [3, 1], F32, kind="ExternalInput")
    outP_d = nc.dram_tensor("outP", [4, 3, NPASS], F32, kind="ExternalOutput")
    outD_d = nc.dram_tensor("outD", [B, 3, DPAD], F32, kind="ExternalOutput")

    ACT_COPY = mybir.ActivationFunctionType.Copy
    ACT_IDENT = mybir.ActivationFunctionType.Identity
    SUB = mybir.AluOpType.subtract
    ABSMAX = mybir.AluOpType.abs_max

    with tile.TileContext(nc) as tc:
        with (
            tc.tile_pool(name="const", bufs=1) as cp,
            tc.tile_pool(name="sb", bufs=2) as sb,
            tc.tile_pool(name="ps", bufs=8, space="PSUM") as ps,
        ):
            wts_t = cp.tile([128, 30], F16)
            nc.sync.dma_start(out=wts_t[:], in_=wts_d[:])
            bias_t = cp.tile([3, 1], F32)
            nc.sync.dma_start(out=bias_t[:], in_=bias_d[:])
            idx_t = cp.tile([128, 8 * NIDX16], I16)
            nc.sync.dma_start(out=idx_t[:], in_=idx_d[:])

            # ---- direct term: out_D[b] = A0 @ x0cm + B0 @ x1cm + bias ----
            DCHUNK = 1536
            for b in range(B):
                for c0 in range(0, DPAD, DCHUNK):
                    xt = sb.tile([128, 2, DCHUNK], F16, tag="xcm")
                    nc.sync.dma_start(out=xt[:],
                                      in_=xcm_d[b, :, :, c0:c0 + DCHUNK])
                    od = sb.tile([3, DCHUNK], F32, tag="od")
                    for (a, w) in _mm_slices(DCHUNK):
                        pt = ps.tile([3, 512], F32, tag="ps")
                        nc.tensor.matmul(pt[:, :w], lhsT=wts_t[:, 0:3],
                                         rhs=xt[:, 0, a:a + w],
                                         start=True, stop=False)
                        nc.tensor.matmul(pt[:, :w], lhsT=wts_t[:, 3:6],
                                         rhs=xt[:, 1, a:a + w],
                                         start=False, stop=True)
                        nc.scalar.activation(od[:, a:a + w], pt[:, :w],
                                             ACT_IDENT, bias=bias_t[:, 0:1])
                    nc.sync.dma_start(out=outD_d[b, :, c0:c0 + DCHUNK],
                                      in_=od[:])

            # ---- gather passes ----
            for b in range(B):
                for p in range(2):
                    j = b * 2 + p
                    cA = 6 + 12 * p      # lin lhsT col for x0-side
                    cB = 9 + 12 * p
                    cA2 = 12 + 12 * p    # abs lhsT col
                    cB2 = 15 + 12 * p
                    for si, segN in enumerate(CAPS):
                        soff = SEG_OFF[si]
                        ha, hb = SEG_HALVES[si]
                        ia0 = (j * 2 + 0) * NIDX16 + soff // 16
                        ib0 = (j * 2 + 1) * NIDX16 + soff // 16
                        g1 = sb.tile([128, 2, segN], F16, tag="g1")
                        g3 = sb.tile([128, 2, segN], F16, tag="g3")
                        nc.gpsimd.dma_gather(
                            g1[:], tabs[(b, ha)][:],
                            idx_t[:, ia0:ia0 + segN // 16],
                            num_idxs=segN, num_idxs_reg=segN,
                            elem_size=256, transpose=True,
                            single_packet=False)
                        nc.gpsimd.dma_gather(
                            g3[:], tabs[(b, hb)][:],
                            idx_t[:, ib0:ib0 + segN // 16],
                            num_idxs=segN, num_idxs_reg=segN,
                            elem_size=256, transpose=True,
                            single_packet=False)
                        dd = sb.tile([128, 2, segN], F16, tag="dd")
                        nc.vector.tensor_tensor(dd[:], g1[:], g3[:], op=SUB)
                        # |d| = max(-d, d) fused as (d * -1) max d
                        nc.vector.scalar_tensor_tensor(
                            dd[:], dd[:], -1.0, dd[:],
                            op0=mybir.AluOpType.mult, op1=mybir.AluOpType.max)
                        og = sb.tile([3, segN], F32, tag="og")
                        for (a, w) in _mm_slices(segN):
                            pt = ps.tile([3, 512], F32, tag="ps")
                            nc.tensor.matmul(pt[:, :w], lhsT=wts_t[:, cA:cA + 3],
                                             rhs=g1[:, 0, a:a + w],
                                             start=True, stop=False)
                            nc.tensor.matmul(pt[:, :w], lhsT=wts_t[:, cA:cA + 3],
                                             rhs=g3[:, 0, a:a + w],
                                             start=False, stop=False)
                            nc.tensor.matmul(pt[:, :w], lhsT=wts_t[:, cB:cB + 3],
                                             rhs=g1[:, 1, a:a + w],
                                             start=False, stop=False)
                            nc.tensor.matmul(pt[:, :w], lhsT=wts_t[:, cB:cB + 3],
                                             rhs=g3[:, 1, a:a + w],
                                             start=False, stop=False)
                            nc.tensor.matmul(pt[:, :w], lhsT=wts_t[:, cA2:cA2 + 3],
                                             rhs=dd[:, 0, a:a + w],
                                             start=False, stop=False)
                            nc.tensor.matmul(pt[:, :w], lhsT=wts_t[:, cB2:cB2 + 3],
                                             rhs=dd[:, 1, a:a + w],
                                             start=False, stop=True)
                            nc.scalar.activation(og[:, a:a + w], pt[:, :w],
                                                 ACT_COPY)
                        nc.sync.dma_start(
                            out=outP_d[j, :, soff:soff + segN], in_=og[:])

    nc.compile()
    return nc


def _wrap_idx(vals):
    """[L] int16 -> wrapped [128, L//16] (i at [i%16, i//16], 8x repl)."""
    w = vals.reshape(-1, 16).T
    return np.tile(w, (8, 1))


def kernel(**inputs):
    global _compiled
    x0 = np.asarray(inputs["x_0"], np.float32)
    x1 = np.asarray(inputs["x_1"], np.float32)
    gemm = np.asarray(inputs["gemm"]).astype(np.int64)

    Wa_local = np.asarray(inputs["Wa_local"], np.float32)
    ba_local = np.asarray(inputs["ba_local"], np.float32)
    Wb_local = np.asarray(inputs["Wb_local"], np.float32)
    bb_local = np.asarray(inputs["bb_local"], np.float32)
    Wa_tri = np.asarray(inputs["Wa_tri"], np.float32)
    ba_tri = np.asarray(inputs["ba_tri"], np.float32)
    Wb_tri = np.asarray(inputs["Wb_tri"], np.float32)
    bb_tri = np.asarray(inputs["bb_tri"], np.float32)
    Wa_fuse = np.asarray(inputs["Wa_fuse"], np.float32)
    ba_fuse = np.asarray(inputs["ba_fuse"], np.float32)
    Wb_fuse = np.asarray(inputs["Wb_fuse"], np.float32)
    bb_fuse = np.asarray(inputs["bb_fuse"], np.float32)

    # ---- fold weights to ten [3,128] effective matrices + bias ----
    Afl, Aft = Wa_fuse[:, :C], Wa_fuse[:, C:]
    Bfl, Bft = Wb_fuse[:, :C], Wb_fuse[:, C:]
    A0 = Afl @ Wa_local + Aft @ Wa_tri[:, :, 0]
    B0 = Bfl @ Wb_local + Bft @ Wb_tri[:, :, 0]
    A1, A2, A3, A4 = (Aft @ Wa_tri[:, :, s] for s in (1, 2, 3, 4))
    B1, B2, B3, B4 = (Bft @ Wb_tri[:, :, s] for s in (1, 2, 3, 4))
    bias = (ba_fuse + bb_fuse + Afl @ ba_local + Aft @ ba_tri
            + Bfl @ bb_local + Bft @ bb_tri)

    mats = [A0, B0, A1, B1, A3, B3, A2, B2, A4, B4]
    wts = np.zeros((128, 30), np.float16)
    for jm, M in enumerate(mats):
        wts[:, 3 * jm:3 * jm + 3] = M.T.astype(np.float16)

    # ---- gather tables (fp16, per-edge rows, lo/hi halves) ----
    tab_in = {}
    for b in range(B):
        tab = np.empty((E, 256), np.float16)
        tab[:, :128] = x0[b].T
        tab[:, 128:] = x1[b].T
        tab_in[f"tab{b}0"] = np.ascontiguousarray(tab[:HALF])
        tab_in[f"tab{b}1"] = np.ascontiguousarray(tab[HALF:])

    # ---- per-core direct shards (channel-major) ----
    xcm = np.zeros((NCORES, B, 128, 2, DPAD), np.float16)
    for k in range(NCORES):
        sl = slice(k * EPC, (k + 1) * EPC)
        for b in range(B):
            xcm[k, b, :, 0, :EPC] = x0[b][:, sl]
            xcm[k, b, :, 1, :EPC] = x1[b][:, sl]

    # ---- pass permutations + wrapped indices ----
    idx_host = np.zeros((NCORES, 128, 8 * NIDX16), np.int16)
    cols_map = np.full((NCORES, B, 2, NPASS), -1, np.int64)
    for b in range(B):
        for p in range(2):
            sA, sB_ = (0, 2) if p == 0 else (1, 3)
            ia, ib = gemm[b, :, sA].copy(), gemm[b, :, sB_].copy()
            swap = (ia >= HALF) & (ib < HALF)
            ia[swap], ib[swap] = ib[swap], ia[swap]
            cls = (ia >= HALF).astype(np.int64) + (ib >= HALF).astype(np.int64)
            j2a = ((b * 2 + p) * 2 + 0) * NIDX16
            j2b = ((b * 2 + p) * 2 + 1) * NIDX16
            for c in range(3):
                edges = np.nonzero(cls == c)[0]
                parts = np.array_split(edges, NCORES)
                cap, soff = CAPS[c], SEG_OFF[c]
                ha, hb = SEG_HALVES[c]
                for k in range(NCORES):
                    el = parts[k]
                    if len(el) > cap:
                        raise RuntimeError(
                            f"class {c} overflow: {len(el)} > {cap}")
                    npad = cap - len(el)
                    cols_map[k, b, p, soff:soff + len(el)] = el
                    iav = np.concatenate(
                        [ia[el] - ha * HALF, np.zeros(npad, np.int64)])
                    ibv = np.concatenate(
                        [ib[el] - hb * HALF, np.zeros(npad, np.int64)])
                    wa = _wrap_idx(iav.astype(np.int16))
                    wb = _wrap_idx(ibv.astype(np.int16))
                    c16 = soff // 16
                    idx_host[k, :, j2a + c16:j2a + c16 + cap // 16] = wa
                    idx_host[k, :, j2b + c16:j2b + c16 + cap // 16] = wb

    # ---- compile once, run ----
    if _compiled is None:
        _compiled = _build_program()
    nc = _compiled

    in_maps = []
    for k in range(NCORES):
        m = dict(tab_in)
        m["xcm"] = xcm[k]
        m["idxs"] = idx_host[k]
        m["wts"] = wts
        m["bias3"] = bias.astype(np.float32).reshape(3, 1)
        in_maps.append(m)

    res = run_bass_kernel_spmd(nc, in_maps, list(range(NCORES)))

    # ---- host assembly ----
    out = np.zeros((B, 3, E), np.float32)
    for k in range(NCORES):
        rD = res.results[k]["outD"]
        rP = res.results[k]["outP"]
        for b in range(B):
            out[b][:, k * EPC:(k + 1) * EPC] += rD[b][:, :EPC]
            for p in range(2):
                cm = cols_map[k, b, p]
                m = cm >= 0
                out[b][:, cm[m]] += rP[b * 2 + p][:, m]
    return out.reshape(B, 1, 3, E)


if __name__ == "__main__":
    rng = np.random.default_rng(0)
    ins = {
        "x_0": rng.standard_normal((B, C, E)).astype(np.float32),
        "x_1": rng.standard_normal((B, C, E)).astype(np.float32),
        "gemm": rng.integers(0, E, (B, E, 4)).astype(np.int32),
        "Wa_local": (rng.standard_normal((C, C)) * 0.05).astype(np.float32),
        "ba_local": (rng.standard_normal(C) * 0.05).astype(np.float32),
        "Wb_local": (rng.standard_normal((C, C)) * 0.05).astype(np.float32),
        "bb_local": (rng.standard_normal(C) * 0.05).astype(np.float32),
        "Wa_tri": (rng.standard_normal((C, C, 5)) * 0.05).astype(np.float32),
        "ba_tri": (rng.standard_normal(C) * 0.05).astype(np.float32),
        "Wb_tri": (rng.standard_normal((C, C, 5)) * 0.05).astype(np.float32),
        "bb_tri": (rng.standard_normal(C) * 0.05).astype(np.float32),
        "Wa_fuse": (rng.standard_normal((3, 2 * C)) * 0.05).astype(np.float32),
        "ba_fuse": (rng.standard_normal(3) * 0.05).astype(np.float32),
        "Wb_fuse": (rng.standard_normal((3, 2 * C)) * 0.05).astype(np.float32),
        "bb_fuse": (rng.standard_normal(3) * 0.05).astype(np.float32),
    }
    y = kernel(**ins)

    # numpy reference
    def np_ref(i):
        o = np.zeros((B, 3, E), np.float32)
        for b in range(B):
            g = i["gemm"][b]
            for x, WL, bL, WT, bT, WF, bF in (
                (i["x_0"][b], i["Wa_local"], i["ba_local"], i["Wa_tri"],
                 i["ba_tri"], i["Wa_fuse"], i["ba_fuse"]),
                (i["x_1"][b], i["Wb_local"], i["bb_local"], i["Wb_tri"],
                 i["bb_tri"], i["Wb_fuse"], i["bb_fuse"]),
            ):
                loc = WL @ x + bL[:, None]
                f = x[:, g]  # [C, E, 4]
                G = np.stack([x, f[..., 0] + f[..., 2], f[..., 1] + f[..., 3],
                              np.abs(f[..., 0] - f[..., 2]),
                              np.abs(f[..., 1] - f[..., 3])], -1)
                tri = np.einsum("ces,ocs->oe", G, WT) + bT[:, None]
                o[b] += WF @ np.concatenate([loc, tri], 0) + bF[:, None]
        return o.reshape(B, 1, 3, E)

    exp = np_ref(ins)
    err = np.abs(y - exp).max() / np.abs(exp).max()
    print("max abs err:", np.abs(y - exp).max(), "rel:", err)



# revision 35
# speedup vs baseline: 443110.1778x; 443110.1778x over previous
"""Trainium2 Bass kernel for the MeshCNN-style GNN message-passing block.

Math: the reference is
    out[b,:,e] = Wa_fuse @ [conv1(x0); mesh_conv(x0)] + Wb_fuse @ [conv1(x1); mesh_conv(x1)] + biases
Everything after the neighbor gather / abs is linear, so the whole block
collapses to ten [3,128] effective matrices applied to:
    x (direct), f1+f3, f2+f4, |f1-f3|, |f2-f4|      (for x0 and x1)
plus one bias 3-vector.  The kernel is gather-dominated (memory regime).

Strategy (8 cores, SPMD, edge-sharded):
- Each core owns E/8 = 7500 contiguous edges.  Its gathers reference at
  most 4*7500 = 30000 distinct rows, so a per-core COMPACTED table
  (dedup'd, host-remapped indices) always fits int16 addressing: one
  dma_gather table per batch, no lo/hi class split, no index classes.
- fp16 rows [x0[b];x1[b]] of 512B; dma_gather(transpose=True) lands
  neighbor features channel-major [128,2,N] for the PE matmul.
- |f1-f3| via DVE subtract + fused (*-1, max).
- The direct (conv1 + s=0) term streams the core's own edge slice
  channel-major (host pre-transposed) and accumulates via PE.
- Host sums outD + outP[p=0] + outP[p=1] per edge slice.
"""

import hashlib
import os
import shutil

import numpy as np

import concourse.bass as bass
import concourse.bacc as bacc
import concourse.tile as tile
from concourse import mybir
from concourse.bass_utils import run_bass_kernel_spmd

# ---- NEFF compile cache keyed on exact BIR bytes so repeat invocations of
# the same program skip neuronxcc. ----
_NEFF_CACHE = os.environ.get("KERNEL_NEFF_CACHE", "/tmp/neff_cache")
try:
    import concourse.bass2jax as _b2j

    if not hasattr(_b2j, "_orig_compile_bir_kernel"):
        _b2j._orig_compile_bir_kernel = _b2j.compile_bir_kernel

        def _cached_compile_bir_kernel(bir_json, tmpdir, neff_name="file.neff"):
            os.makedirs(_NEFF_CACHE, exist_ok=True)
            key = hashlib.sha256(bir_json).hexdigest()
            cpath = os.path.join(_NEFF_CACHE, key + ".neff")
            out = os.path.join(tmpdir, neff_name)
            if os.path.exists(cpath):
                shutil.copyfile(cpath, out)
                return out
            path = _b2j._orig_compile_bir_kernel(bir_json, tmpdir, neff_name)
            tmp = cpath + ".tmp"
            shutil.copyfile(path, tmp)
            os.replace(tmp, cpath)
            return path

        _b2j.compile_bir_kernel = _cached_compile_bir_kernel

    _b2j.install_neuronx_cc_hook()
    import libneuronxla as _lnx

    if hasattr(_lnx, "orig_neuronx_cc") and not hasattr(_lnx, "_ant_cc_cached"):
        _lnx._ant_cc_cached = True
        _orig_cc = _lnx.orig_neuronx_cc

        def _cached_cc(code, code_format, platform_version, file_prefix):
            os.makedirs(_NEFF_CACHE, exist_ok=True)
            key = hashlib.sha256(
                bytes(code) + bytes(code_format) + str(platform_version).encode()
            ).hexdigest()
            cpath = os.path.join(_NEFF_CACHE, key + ".cc")
            if os.path.exists(cpath):
                with open(cpath, "rb") as f:
                    return 0, f.read()
            r = _orig_cc(code, code_format, platform_version, file_prefix)
            try:
                rc, blob = r
                if rc == 0 and isinstance(blob, (bytes, bytearray)):
                    tmp = cpath + ".tmp"
                    with open(tmp, "wb") as f:
                        f.write(blob)
                    os.replace(tmp, cpath)
            except Exception:
                pass
            return r

        _lnx.orig_neuronx_cc = _cached_cc
except Exception:
    pass

B, C, E = 2, 128, 60000
NCORES = 8
EPC = E // NCORES              # 7500 edges per core
NP = 7552                      # padded edge count
CHS = (3840, 3712)             # gather chunk sizes (each a multiple of 128)
CHOFF = (0, 3840)              # chunk column offsets
NTAB = 30016                   # static compact-table rows (>= 4*7500)

F16 = mybir.dt.float16
F32 = mybir.dt.float32
I16 = mybir.dt.int16

_compiled = {}


def _mm_slices(n):
    """Split [0,n) into PSUM-bank-aligned matmul slices of <=512."""
    out = []
    a = 0
    while a < n:
        w = min(512, n - a)
        out.append((a, w))
        a += w
    return out


def _build_program(reps=1, nq=4, scratch=16384, qstride=2,
                   do_gather=True, do_trans=True, do_dve=True, do_mm=True):
    # Multi-queue SWDGE quadruples gather descriptor-generation throughput
    # (each queue runs on its own Q7 cpu pair), but the XBAR-transpose spray
    # path is not multi-queue safe (concurrent sprays corrupt data), so the
    # gathers land row-major (transpose=False) and the PE transposes blocks
    # into PSUM via is_transpose matmuls against an identity matrix.  DVE
    # then computes s = f1+f3 and |f1-f3| straight from PSUM into SBUF,
    # which also halves the downstream matmul count.
    nc = bacc.Bacc("TRN2", target_bir_lowering=False, debug=False,
                   num_devices=NCORES, num_swdge_queues=nq,
                   dynamic_dma_scratch_size=scratch)
    swdge_emit = [0]

    def next_q():
        q = swdge_emit[0] % nq
        swdge_emit[0] += 1
        return q

    tabs = [nc.dram_tensor(f"tab{b}", [NTAB, 256], F16, kind="ExternalInput")
            for b in range(B)]
    xcm_d = nc.dram_tensor("xcm", [B, 128, 2, NP], F16, kind="ExternalInput")
    idx_d = nc.dram_tensor("idxs", [128, NP // 2], I16, kind="ExternalInput")
    wts_d = nc.dram_tensor("wts", [128, 30], F16, kind="ExternalInput")
    bias_d = nc.dram_tensor("bias3", [3, 1], F32, kind="ExternalInput")
    idn_d = nc.dram_tensor("idn", [128, 128], F16, kind="ExternalInput")
    outP_d = nc.dram_tensor("outP", [4, 3, NP], F32, kind="ExternalOutput")
    outD_d = nc.dram_tensor("outD", [B, 3, NP], F32, kind="ExternalOutput")

    ACT_COPY = mybir.ActivationFunctionType.Copy
    ACT_IDENT = mybir.ActivationFunctionType.Identity
    SUB = mybir.AluOpType.subtract
    ADD = mybir.AluOpType.add

    with tile.TileContext(nc) as tc:
        with (
            tc.tile_pool(name="const", bufs=1) as cp,
            tc.tile_pool(name="sb", bufs=2) as sb,
            tc.tile_pool(name="ps", bufs=4, space="PSUM") as ps,
            tc.tile_pool(name="pst", bufs=2, space="PSUM") as pst,
        ):
            wts_t = cp.tile([128, 30], F16)
            nc.sync.dma_start(out=wts_t[:], in_=wts_d[:])
            bias_t = cp.tile([3, 1], F32)
            nc.sync.dma_start(out=bias_t[:], in_=bias_d[:])
            idx_t = cp.tile([128, NP // 2], I16)
            nc.sync.dma_start(out=idx_t[:], in_=idx_d[:])
            idn_t = cp.tile([128, 128], F16)
            nc.sync.dma_start(out=idn_t[:], in_=idn_d[:])

            for _rep in range(reps):
                for b in range(B):
                    # ---- direct term: A0 @ x0cm + B0 @ x1cm + bias ----
                    for c0, cw in zip(CHOFF, CHS):
                        xt = sb.tile([128, 2, cw], F16, tag="xt")
                        nc.sync.dma_start(out=xt[:],
                                          in_=xcm_d[b, :, :, c0:c0 + cw])
                        od = sb.tile([3, cw], F32, tag="od")
                        for (a, w) in _mm_slices(cw):
                            pt = ps.tile([3, 512], F32, tag="ps")
                            nc.tensor.matmul(pt[:, :w], lhsT=wts_t[:, 0:3],
                                             rhs=xt[:, 0, a:a + w],
                                             start=True, stop=False)
                            nc.tensor.matmul(pt[:, :w], lhsT=wts_t[:, 3:6],
                                             rhs=xt[:, 1, a:a + w],
                                             start=False, stop=True)
                            nc.scalar.activation(od[:, a:a + w], pt[:, :w],
                                                 ACT_IDENT, bias=bias_t[:, 0:1])
                        nc.sync.dma_start(out=outD_d[b, :, c0:c0 + cw],
                                          in_=od[:])

                    # ---- gather passes ----
                    for p in range(2):
                        j = b * 2 + p
                        cA = 6 + 12 * p      # lin lhsT col for x0-side
                        cB = 9 + 12 * p
                        cA2 = 12 + 12 * p    # abs lhsT col
                        cB2 = 15 + 12 * p
                        for ci, (c0, cw) in enumerate(zip(CHOFF, CHS)):
                            ia0 = (j * 2 + 0) * (NP // 16) + c0 // 16
                            ib0 = (j * 2 + 1) * (NP // 16) + c0 // 16
                            g1 = sb.tile([128, cw // 128, 256], F16, tag="g1",
                                         bufs=3)
                            g3 = sb.tile([128, cw // 128, 256], F16, tag="g3",
                                         bufs=3)
                            if not do_gather:
                                nc.scalar.activation(g1[0:3, 0, 0:4],
                                                     wts_t[0:3, 0:4], ACT_COPY)
                                nc.scalar.activation(g3[0:3, 0, 0:4],
                                                     wts_t[0:3, 0:4], ACT_COPY)
                            if do_gather:
                                nc.gpsimd.dma_gather(
                                    g1[:], tabs[b][:],
                                    idx_t[:, ia0:ia0 + cw // 16],
                                    num_idxs=cw, num_idxs_reg=cw,
                                    elem_size=256, transpose=False,
                                    single_packet=False, queue_num=next_q())
                                nc.gpsimd.dma_gather(
                                    g3[:], tabs[b][:],
                                    idx_t[:, ib0:ib0 + cw // 16],
                                    num_idxs=cw, num_idxs_reg=cw,
                                    elem_size=256, transpose=False,
                                    single_packet=False, queue_num=next_q())
                            og = sb.tile([3, cw], F32, tag="og")
                            for (a, w) in _mm_slices(cw):
                                g1t = pst.tile([128, 2, 512], F16, tag="pt1")
                                g3t = pst.tile([128, 2, 512], F16, tag="pt2")
                                if do_trans:
                                    for kb in range(w // 128):
                                        eb = (a + kb * 128) // 128
                                        for chh in range(2):
                                            nc.tensor.matmul(
                                                g1t[:, chh, kb * 128:kb * 128 + 128],
                                                lhsT=g1[:, eb, chh * 128:chh * 128 + 128],
                                                rhs=idn_t[:],
                                                start=True, stop=True,
                                                is_transpose=True)
                                            nc.tensor.matmul(
                                                g3t[:, chh, kb * 128:kb * 128 + 128],
                                                lhsT=g3[:, eb, chh * 128:chh * 128 + 128],
                                                rhs=idn_t[:],
                                                start=True, stop=True,
                                                is_transpose=True)
                                g1s = sb.tile([128, 2, 512], F16, tag="g1s", bufs=3)
                                s = sb.tile([128, 2, 512], F16, tag="s", bufs=3)
                                dd = sb.tile([128, 2, 512], F16, tag="dd", bufs=3)
                                if not do_dve and do_mm:
                                    nc.scalar.activation(s[0:3, 0, 0:4],
                                                         wts_t[0:3, 0:4],
                                                         ACT_COPY)
                                    nc.scalar.activation(dd[0:3, 0, 0:4],
                                                         wts_t[0:3, 0:4],
                                                         ACT_COPY)
                                if do_dve:
                                    nc.scalar.activation(g1s[:, :, :w],
                                                         g1t[:, :, :w],
                                                         ACT_COPY)
                                    nc.vector.tensor_tensor(
                                        s[:, :, :w], g1s[:, :, :w],
                                        g3t[:, :, :w], op=ADD)
                                    nc.vector.tensor_tensor(
                                        dd[:, :, :w], g1s[:, :, :w],
                                        g3t[:, :, :w], op=SUB)
                                    # |d| = max(d * -1, d)
                                    nc.vector.scalar_tensor_tensor(
                                        dd[:, :, :w], dd[:, :, :w], -1.0,
                                        dd[:, :, :w],
                                        op0=mybir.AluOpType.mult,
                                        op1=mybir.AluOpType.max)
                                if do_mm:
                                    pt = ps.tile([3, 512], F32, tag="ps")
                                    nc.tensor.matmul(pt[:, :w],
                                                     lhsT=wts_t[:, cA:cA + 3],
                                                     rhs=s[:, 0, :w],
                                                     start=True, stop=False)
                                    nc.tensor.matmul(pt[:, :w],
                                                     lhsT=wts_t[:, cB:cB + 3],
                                                     rhs=s[:, 1, :w],
                                                     start=False, stop=False)
                                    nc.tensor.matmul(pt[:, :w],
                                                     lhsT=wts_t[:, cA2:cA2 + 3],
                                                     rhs=dd[:, 0, :w],
                                                     start=False, stop=False)
                                    nc.tensor.matmul(pt[:, :w],
                                                     lhsT=wts_t[:, cB2:cB2 + 3],
                                                     rhs=dd[:, 1, :w],
                                                     start=False, stop=True)
                                    nc.scalar.activation(og[:, a:a + w],
                                                         pt[:, :w], ACT_COPY)
                            if not do_mm:
                                nc.scalar.activation(og[0:3, 0:4],
                                                     wts_t[0:3, 0:4], ACT_COPY)
                            nc.sync.dma_start(
                                out=outP_d[j, :, c0:c0 + cw],
                                in_=og[:])

    nc.compile()
    return nc


def _wrap_idx(vals):
    """[L] int16 -> wrapped [128, L//16] (i at [i%16, i//16], 8x repl)."""
    w = vals.reshape(-1, 16).T
    return np.tile(w, (8, 1))


def _prep_inputs(inputs):
    """Fold weights + build per-core compact tables, indices, shards."""
    x0 = np.asarray(inputs["x_0"], np.float32)
    x1 = np.asarray(inputs["x_1"], np.float32)
    gemm = np.asarray(inputs["gemm"]).astype(np.int64)

    Wa_local = np.asarray(inputs["Wa_local"], np.float32)
    ba_local = np.asarray(inputs["ba_local"], np.float32)
    Wb_local = np.asarray(inputs["Wb_local"], np.float32)
    bb_local = np.asarray(inputs["bb_local"], np.float32)
    Wa_tri = np.asarray(inputs["Wa_tri"], np.float32)
    ba_tri = np.asarray(inputs["ba_tri"], np.float32)
    Wb_tri = np.asarray(inputs["Wb_tri"], np.float32)
    bb_tri = np.asarray(inputs["bb_tri"], np.float32)
    Wa_fuse = np.asarray(inputs["Wa_fuse"], np.float32)
    ba_fuse = np.asarray(inputs["ba_fuse"], np.float32)
    Wb_fuse = np.asarray(inputs["Wb_fuse"], np.float32)
    bb_fuse = np.asarray(inputs["bb_fuse"], np.float32)

    # ---- fold weights to ten [3,128] effective matrices + bias ----
    Afl, Aft = Wa_fuse[:, :C], Wa_fuse[:, C:]
    Bfl, Bft = Wb_fuse[:, :C], Wb_fuse[:, C:]
    A0 = Afl @ Wa_local + Aft @ Wa_tri[:, :, 0]
    B0 = Bfl @ Wb_local + Bft @ Wb_tri[:, :, 0]
    A1, A2, A3, A4 = (Aft @ Wa_tri[:, :, s] for s in (1, 2, 3, 4))
    B1, B2, B3, B4 = (Bft @ Wb_tri[:, :, s] for s in (1, 2, 3, 4))
    bias = (ba_fuse + bb_fuse + Afl @ ba_local + Aft @ ba_tri
            + Bfl @ bb_local + Bft @ bb_tri)

    mats = [A0, B0, A1, B1, A3, B3, A2, B2, A4, B4]
    wts = np.zeros((128, 30), np.float16)
    for jm, M in enumerate(mats):
        wts[:, 3 * jm:3 * jm + 3] = M.T.astype(np.float16)

    # ---- master per-batch tables: row e = [x0[b,:,e]; x1[b,:,e]] fp16 ----
    mtab = np.empty((B, E, 256), np.float16)
    for b in range(B):
        mtab[b, :, :128] = x0[b].T
        mtab[b, :, 128:] = x1[b].T

    in_maps = []
    for k in range(NCORES):
        sl = slice(k * EPC, (k + 1) * EPC)
        m = {"wts": wts, "bias3": bias.astype(np.float32).reshape(3, 1),
             "idn": np.eye(128, dtype=np.float16)}

        xcm = np.zeros((B, 128, 2, NP), np.float16)
        idx_host = np.zeros((128, NP // 2), np.int16)
        for b in range(B):
            xcm[b, :, 0, :EPC] = x0[b][:, sl]
            xcm[b, :, 1, :EPC] = x1[b][:, sl]

            g = gemm[b, sl]                    # [EPC, 4]
            uniq = np.unique(g)                # <= 30000 rows
            tabk = np.zeros((NTAB, 256), np.float16)
            tabk[:len(uniq)] = mtab[b][uniq]
            m[f"tab{b}"] = tabk
            remap = np.searchsorted(uniq, g).astype(np.int64)  # [EPC, 4]
            for p in range(2):
                j = b * 2 + p
                for slot, col in ((0, p), (1, p + 2)):
                    v = np.zeros(NP, np.int16)
                    v[:EPC] = remap[:, col]
                    base = (j * 2 + slot) * (NP // 16)
                    idx_host[:, base:base + NP // 16] = _wrap_idx(v)
        m["xcm"] = xcm
        m["idxs"] = idx_host
        in_maps.append(m)
    return in_maps


def _assemble(results):
    out = np.zeros((B, 3, E), np.float32)
    for k in range(NCORES):
        sl = slice(k * EPC, (k + 1) * EPC)
        rD = results[k]["outD"]
        rP = results[k]["outP"]
        for b in range(B):
            out[b][:, sl] = (rD[b][:, :EPC]
                             + rP[b * 2 + 0][:, :EPC]
                             + rP[b * 2 + 1][:, :EPC])
    return out.reshape(B, 1, 3, E)


def kernel(**inputs):
    in_maps = _prep_inputs(inputs)
    if 1 not in _compiled:
        _compiled[1] = _build_program(1)
    nc = _compiled[1]
    res = run_bass_kernel_spmd(nc, in_maps, list(range(NCORES)))
    return _assemble(res.results)


if __name__ == "__main__":
    rng = np.random.default_rng(0)
    ins = {
        "x_0": rng.standard_normal((B, C, E)).astype(np.float32),
        "x_1": rng.standard_normal((B, C, E)).astype(np.float32),
        "gemm": rng.integers(0, E, (B, E, 4)).astype(np.int32),
        "Wa_local": (rng.standard_normal((C, C)) * 0.05).astype(np.float32),
        "ba_local": (rng.standard_normal(C) * 0.05).astype(np.float32),
        "Wb_local": (rng.standard_normal((C, C)) * 0.05).astype(np.float32),
        "bb_local": (rng.standard_normal(C) * 0.05).astype(np.float32),
        "Wa_tri": (rng.standard_normal((C, C, 5)) * 0.05).astype(np.float32),
        "ba_tri": (rng.standard_normal(C) * 0.05).astype(np.float32),
        "Wb_tri": (rng.standard_normal((C, C, 5)) * 0.05).astype(np.float32),
        "bb_tri": (rng.standard_normal(C) * 0.05).astype(np.float32),
        "Wa_fuse": (rng.standard_normal((3, 2 * C)) * 0.05).astype(np.float32),
        "ba_fuse": (rng.standard_normal(3) * 0.05).astype(np.float32),
        "Wb_fuse": (rng.standard_normal((3, 2 * C)) * 0.05).astype(np.float32),
        "bb_fuse": (rng.standard_normal(3) * 0.05).astype(np.float32),
    }
    y = kernel(**ins)

    def np_ref(i):
        o = np.zeros((B, 3, E), np.float32)
        for b in range(B):
            g = i["gemm"][b]
            for x, WL, bL, WT, bT, WF, bF in (
                (i["x_0"][b], i["Wa_local"], i["ba_local"], i["Wa_tri"],
                 i["ba_tri"], i["Wa_fuse"], i["ba_fuse"]),
                (i["x_1"][b], i["Wb_local"], i["bb_local"], i["Wb_tri"],
                 i["bb_tri"], i["Wb_fuse"], i["bb_fuse"]),
            ):
                loc = WL @ x + bL[:, None]
                f = x[:, g]  # [C, E, 4]
                G = np.stack([x, f[..., 0] + f[..., 2], f[..., 1] + f[..., 3],
                              np.abs(f[..., 0] - f[..., 2]),
                              np.abs(f[..., 1] - f[..., 3])], -1)
                tri = np.einsum("ces,ocs->oe", G, WT) + bT[:, None]
                o[b] += WF @ np.concatenate([loc, tri], 0) + bF[:, None]
        return o.reshape(B, 1, 3, E)

    exp = np_ref(ins)
    err = np.abs(y - exp).max() / np.abs(exp).max()
    print("max abs err:", np.abs(y - exp).max(), "rel:", err)


# revision 38
# speedup vs baseline: 483810.6077x; 1.0919x over previous
"""Trainium2 Bass kernel for the MeshCNN-style GNN message-passing block.

Math: the reference is
    out[b,:,e] = Wa_fuse @ [conv1(x0); mesh_conv(x0)] + Wb_fuse @ [conv1(x1); mesh_conv(x1)] + biases
Everything after the neighbor gather / abs is linear, so the whole block
collapses to ten [3,128] effective matrices applied to:
    x (direct), f1+f3, f2+f4, |f1-f3|, |f2-f4|      (for x0 and x1)
plus one bias 3-vector.  The kernel is gather-dominated (memory regime).

Strategy (8 cores, SPMD, edge-sharded):
- Each core owns E/8 = 7500 contiguous edges.  Its gathers reference at
  most 4*7500 = 30000 distinct rows, so a per-core COMPACTED table
  (dedup'd, host-remapped indices) always fits int16 addressing: one
  dma_gather table per batch, no lo/hi class split, no index classes.
- fp16 rows [x0[b];x1[b]] of 512B; dma_gather(transpose=True) lands
  neighbor features channel-major [128,2,N] for the PE matmul.
- |f1-f3| via DVE subtract + fused (*-1, max).
- The direct (conv1 + s=0) term streams the core's own edge slice
  channel-major (host pre-transposed) and accumulates via PE.
- Host sums outD + outP[p=0] + outP[p=1] per edge slice.
"""

import hashlib
import os
import shutil

import numpy as np

import concourse.bass as bass
import concourse.bacc as bacc
import concourse.tile as tile
from concourse import mybir
from concourse.bass_utils import run_bass_kernel_spmd

# ---- NEFF compile cache keyed on exact BIR bytes so repeat invocations of
# the same program skip neuronxcc. ----
_NEFF_CACHE = os.environ.get("KERNEL_NEFF_CACHE", "/tmp/neff_cache")
try:
    import concourse.bass2jax as _b2j

    if not hasattr(_b2j, "_orig_compile_bir_kernel"):
        _b2j._orig_compile_bir_kernel = _b2j.compile_bir_kernel

        def _cached_compile_bir_kernel(bir_json, tmpdir, neff_name="file.neff"):
            os.makedirs(_NEFF_CACHE, exist_ok=True)
            key = hashlib.sha256(bir_json).hexdigest()
            cpath = os.path.join(_NEFF_CACHE, key + ".neff")
            out = os.path.join(tmpdir, neff_name)
            if os.path.exists(cpath):
                shutil.copyfile(cpath, out)
                return out
            path = _b2j._orig_compile_bir_kernel(bir_json, tmpdir, neff_name)
            tmp = cpath + ".tmp"
            shutil.copyfile(path, tmp)
            os.replace(tmp, cpath)
            return path

        _b2j.compile_bir_kernel = _cached_compile_bir_kernel

    _b2j.install_neuronx_cc_hook()
    import libneuronxla as _lnx

    if hasattr(_lnx, "orig_neuronx_cc") and not hasattr(_lnx, "_ant_cc_cached"):
        _lnx._ant_cc_cached = True
        _orig_cc = _lnx.orig_neuronx_cc

        def _cached_cc(code, code_format, platform_version, file_prefix):
            os.makedirs(_NEFF_CACHE, exist_ok=True)
            key = hashlib.sha256(
                bytes(code) + bytes(code_format) + str(platform_version).encode()
            ).hexdigest()
            cpath = os.path.join(_NEFF_CACHE, key + ".cc")
            if os.path.exists(cpath):
                with open(cpath, "rb") as f:
                    return 0, f.read()
            r = _orig_cc(code, code_format, platform_version, file_prefix)
            try:
                rc, blob = r
                if rc == 0 and isinstance(blob, (bytes, bytearray)):
                    tmp = cpath + ".tmp"
                    with open(tmp, "wb") as f:
                        f.write(blob)
                    os.replace(tmp, cpath)
            except Exception:
                pass
            return r

        _lnx.orig_neuronx_cc = _cached_cc
except Exception:
    pass

B, C, E = 2, 128, 60000
NCORES = 8
EPC = E // NCORES              # 7500 edges per core
NP = 7552                      # padded edge count
CHS = (3840, 3712)             # gather chunk sizes (each a multiple of 128)
CHOFF = (0, 3840)              # chunk column offsets
NTAB = 30016                   # static compact-table rows (>= 4*7500)

F16 = mybir.dt.float16
F32 = mybir.dt.float32
I16 = mybir.dt.int16

_compiled = {}


def _mm_slices(n):
    """Split [0,n) into PSUM-bank-aligned matmul slices of <=512."""
    out = []
    a = 0
    while a < n:
        w = min(512, n - a)
        out.append((a, w))
        a += w
    return out


def _build_program(reps=1, nq=4, scratch=16384, qstride=2,
                   do_gather=True, do_trans=True, do_dve=True, do_mm=True):
    # Multi-queue SWDGE quadruples gather descriptor-generation throughput
    # (each queue runs on its own Q7 cpu pair), but the XBAR-transpose spray
    # path is not multi-queue safe (concurrent sprays corrupt data), so the
    # gathers land row-major (transpose=False) and the PE transposes blocks
    # into PSUM via is_transpose matmuls against an identity matrix.  DVE
    # then computes s = f1+f3 and |f1-f3| straight from PSUM into SBUF,
    # which also halves the downstream matmul count.
    nc = bacc.Bacc("TRN2", target_bir_lowering=False, debug=False,
                   num_devices=NCORES, num_swdge_queues=nq,
                   dynamic_dma_scratch_size=scratch)
    swdge_emit = [0]

    def next_q():
        q = swdge_emit[0] % nq
        swdge_emit[0] += 1
        return q

    tabs = [nc.dram_tensor(f"tab{b}", [NTAB, 256], F16, kind="ExternalInput")
            for b in range(B)]
    xcm_d = nc.dram_tensor("xcm", [B, 128, 2, NP], F16, kind="ExternalInput")
    idx_d = nc.dram_tensor("idxs", [128, NP // 2], I16, kind="ExternalInput")
    wts_d = nc.dram_tensor("wts", [128, 30], F16, kind="ExternalInput")
    bias_d = nc.dram_tensor("bias3", [3, 1], F32, kind="ExternalInput")
    idn_d = nc.dram_tensor("idn", [128, 128], F16, kind="ExternalInput")
    outP_d = nc.dram_tensor("outP", [4, 3, NP], F32, kind="ExternalOutput")
    outD_d = nc.dram_tensor("outD", [B, 3, NP], F32, kind="ExternalOutput")

    ACT_COPY = mybir.ActivationFunctionType.Copy
    ACT_IDENT = mybir.ActivationFunctionType.Identity
    SUB = mybir.AluOpType.subtract
    ADD = mybir.AluOpType.add

    with tile.TileContext(nc) as tc:
        with (
            tc.tile_pool(name="const", bufs=1) as cp,
            tc.tile_pool(name="sb", bufs=2) as sb,
            tc.tile_pool(name="ps", bufs=2, space="PSUM") as ps,
            tc.tile_pool(name="pst", bufs=3, space="PSUM") as pst,
        ):
            wts_t = cp.tile([128, 30], F16)
            nc.sync.dma_start(out=wts_t[:], in_=wts_d[:])
            bias_t = cp.tile([3, 1], F32)
            nc.sync.dma_start(out=bias_t[:], in_=bias_d[:])
            idx_t = cp.tile([128, NP // 2], I16)
            nc.sync.dma_start(out=idx_t[:], in_=idx_d[:])
            idn_t = cp.tile([128, 128], F16)
            nc.sync.dma_start(out=idn_t[:], in_=idn_d[:])

            for _rep in range(reps):
                for b in range(B):
                    # ---- direct term: A0 @ x0cm + B0 @ x1cm + bias ----
                    for c0, cw in zip(CHOFF, CHS):
                        xt = sb.tile([128, 2, cw], F16, tag="xt")
                        nc.sync.dma_start(out=xt[:],
                                          in_=xcm_d[b, :, :, c0:c0 + cw])
                        od = sb.tile([3, cw], F32, tag="od")
                        for (a, w) in _mm_slices(cw):
                            pt = ps.tile([3, 512], F32, tag="ps")
                            nc.tensor.matmul(pt[:, :w], lhsT=wts_t[:, 0:3],
                                             rhs=xt[:, 0, a:a + w],
                                             start=True, stop=False)
                            nc.tensor.matmul(pt[:, :w], lhsT=wts_t[:, 3:6],
                                             rhs=xt[:, 1, a:a + w],
                                             start=False, stop=True)
                            nc.scalar.activation(od[:, a:a + w], pt[:, :w],
                                                 ACT_IDENT, bias=bias_t[:, 0:1])
                        nc.sync.dma_start(out=outD_d[b, :, c0:c0 + cw],
                                          in_=od[:])

                    # ---- gather passes ----
                    for p in range(2):
                        j = b * 2 + p
                        cA = 6 + 12 * p      # lin lhsT col for x0-side
                        cB = 9 + 12 * p
                        cA2 = 12 + 12 * p    # abs lhsT col
                        cB2 = 15 + 12 * p
                        for ci, (c0, cw) in enumerate(zip(CHOFF, CHS)):
                            i0 = j * (2 * NP // 16) + 2 * c0 // 16
                            g13 = sb.tile([128, 2 * cw // 128, 256], F16,
                                          tag="g13", bufs=3)
                            nc.gpsimd.dma_gather(
                                g13[:], tabs[b][:],
                                idx_t[:, i0:i0 + 2 * cw // 16],
                                num_idxs=2 * cw, num_idxs_reg=2 * cw,
                                elem_size=256, transpose=False,
                                single_packet=False, queue_num=next_q())
                            g1 = g13[:, 0:cw // 128, :]
                            g3 = g13[:, cw // 128:2 * cw // 128, :]
                            og = sb.tile([3, cw], F32, tag="og")
                            for (a, w) in _mm_slices(cw):
                                g1t = pst.tile([128, 2, 512], F16, tag="pt1")
                                g3t = pst.tile([128, 2, 512], F16, tag="pt2")
                                if do_trans:
                                    for kb in range(w // 128):
                                        eb = (a + kb * 128) // 128
                                        for chh in range(2):
                                            nc.tensor.matmul(
                                                g1t[:, chh, kb * 128:kb * 128 + 128],
                                                lhsT=g1[:, eb, chh * 128:chh * 128 + 128],
                                                rhs=idn_t[:],
                                                start=True, stop=True,
                                                is_transpose=True)
                                            nc.tensor.matmul(
                                                g3t[:, chh, kb * 128:kb * 128 + 128],
                                                lhsT=g3[:, eb, chh * 128:chh * 128 + 128],
                                                rhs=idn_t[:],
                                                start=True, stop=True,
                                                is_transpose=True)
                                g1s = sb.tile([128, 2, 512], F16, tag="g1s", bufs=5)
                                s = sb.tile([128, 2, 512], F16, tag="s", bufs=5)
                                dd = sb.tile([128, 2, 512], F16, tag="dd", bufs=5)
                                if not do_dve and do_mm:
                                    nc.scalar.activation(s[0:3, 0, 0:4],
                                                         wts_t[0:3, 0:4],
                                                         ACT_COPY)
                                    nc.scalar.activation(dd[0:3, 0, 0:4],
                                                         wts_t[0:3, 0:4],
                                                         ACT_COPY)
                                if do_dve:
                                    nc.scalar.activation(g1s[:, :, :w],
                                                         g1t[:, :, :w],
                                                         ACT_COPY)
                                    nc.vector.tensor_tensor(
                                        s[:, :, :w], g1s[:, :, :w],
                                        g3t[:, :, :w], op=ADD)
                                    nc.vector.tensor_tensor(
                                        dd[:, :, :w], g1s[:, :, :w],
                                        g3t[:, :, :w], op=SUB)
                                    # |d| = max(d * -1, d)
                                    nc.vector.scalar_tensor_tensor(
                                        dd[:, :, :w], dd[:, :, :w], -1.0,
                                        dd[:, :, :w],
                                        op0=mybir.AluOpType.mult,
                                        op1=mybir.AluOpType.max)
                                if do_mm:
                                    pt = ps.tile([3, 512], F32, tag="ps")
                                    nc.tensor.matmul(pt[:, :w],
                                                     lhsT=wts_t[:, cA:cA + 3],
                                                     rhs=s[:, 0, :w],
                                                     start=True, stop=False)
                                    nc.tensor.matmul(pt[:, :w],
                                                     lhsT=wts_t[:, cB:cB + 3],
                                                     rhs=s[:, 1, :w],
                                                     start=False, stop=False)
                                    nc.tensor.matmul(pt[:, :w],
                                                     lhsT=wts_t[:, cA2:cA2 + 3],
                                                     rhs=dd[:, 0, :w],
                                                     start=False, stop=False)
                                    nc.tensor.matmul(pt[:, :w],
                                                     lhsT=wts_t[:, cB2:cB2 + 3],
                                                     rhs=dd[:, 1, :w],
                                                     start=False, stop=True)
                                    nc.scalar.activation(og[:, a:a + w],
                                                         pt[:, :w], ACT_COPY)
                            if not do_mm:
                                nc.scalar.activation(og[0:3, 0:4],
                                                     wts_t[0:3, 0:4], ACT_COPY)
                            nc.sync.dma_start(
                                out=outP_d[j, :, c0:c0 + cw],
                                in_=og[:])

    # Post-schedule queue assignment: Tile round-robins SWDGE completions
    # over 8 DMASW sems in SCHEDULED order and its cumulative thresholds
    # assume in-order completion per sem, which only holds when all gathers
    # sharing a sem run on one FIFO HW queue.  queue_num is mutable after
    # scheduling, so spread sems over the 4 queues by row-count load.
    sem_groups = {}
    for blk in nc.m.functions[0].blocks:
        for inst in blk.instructions:
            if isinstance(inst, mybir.InstDMAGatherAnt) and inst.sync_info:
                sem_groups.setdefault(inst.sync_info.on_update[0].id,
                                      []).append(inst)
    loads = [0, 0, 0, 0]
    for load, insts in sorted(
            ((sum(i.num_idxs for i in g), g) for g in sem_groups.values()),
            key=lambda t: -t[0]):
        q = min(range(4), key=lambda k: loads[k])
        loads[q] += load
        for i in insts:
            i.queue_num = q

    nc.compile()
    return nc


def _wrap_idx(vals):
    """[L] int16 -> wrapped [128, L//16] (i at [i%16, i//16], 8x repl)."""
    w = vals.reshape(-1, 16).T
    return np.tile(w, (8, 1))


def _prep_inputs(inputs):
    """Fold weights + build per-core compact tables, indices, shards."""
    x0 = np.asarray(inputs["x_0"], np.float32)
    x1 = np.asarray(inputs["x_1"], np.float32)
    gemm = np.asarray(inputs["gemm"]).astype(np.int64)

    Wa_local = np.asarray(inputs["Wa_local"], np.float32)
    ba_local = np.asarray(inputs["ba_local"], np.float32)
    Wb_local = np.asarray(inputs["Wb_local"], np.float32)
    bb_local = np.asarray(inputs["bb_local"], np.float32)
    Wa_tri = np.asarray(inputs["Wa_tri"], np.float32)
    ba_tri = np.asarray(inputs["ba_tri"], np.float32)
    Wb_tri = np.asarray(inputs["Wb_tri"], np.float32)
    bb_tri = np.asarray(inputs["bb_tri"], np.float32)
    Wa_fuse = np.asarray(inputs["Wa_fuse"], np.float32)
    ba_fuse = np.asarray(inputs["ba_fuse"], np.float32)
    Wb_fuse = np.asarray(inputs["Wb_fuse"], np.float32)
    bb_fuse = np.asarray(inputs["bb_fuse"], np.float32)

    # ---- fold weights to ten [3,128] effective matrices + bias ----
    Afl, Aft = Wa_fuse[:, :C], Wa_fuse[:, C:]
    Bfl, Bft = Wb_fuse[:, :C], Wb_fuse[:, C:]
    A0 = Afl @ Wa_local + Aft @ Wa_tri[:, :, 0]
    B0 = Bfl @ Wb_local + Bft @ Wb_tri[:, :, 0]
    A1, A2, A3, A4 = (Aft @ Wa_tri[:, :, s] for s in (1, 2, 3, 4))
    B1, B2, B3, B4 = (Bft @ Wb_tri[:, :, s] for s in (1, 2, 3, 4))
    bias = (ba_fuse + bb_fuse + Afl @ ba_local + Aft @ ba_tri
            + Bfl @ bb_local + Bft @ bb_tri)

    mats = [A0, B0, A1, B1, A3, B3, A2, B2, A4, B4]
    wts = np.zeros((128, 30), np.float16)
    for jm, M in enumerate(mats):
        wts[:, 3 * jm:3 * jm + 3] = M.T.astype(np.float16)

    # ---- master per-batch tables: row e = [x0[b,:,e]; x1[b,:,e]] fp16 ----
    mtab = np.empty((B, E, 256), np.float16)
    for b in range(B):
        mtab[b, :, :128] = x0[b].T
        mtab[b, :, 128:] = x1[b].T

    in_maps = []
    for k in range(NCORES):
        sl = slice(k * EPC, (k + 1) * EPC)
        m = {"wts": wts, "bias3": bias.astype(np.float32).reshape(3, 1),
             "idn": np.eye(128, dtype=np.float16)}

        xcm = np.zeros((B, 128, 2, NP), np.float16)
        idx_host = np.zeros((128, NP // 2), np.int16)
        for b in range(B):
            xcm[b, :, 0, :EPC] = x0[b][:, sl]
            xcm[b, :, 1, :EPC] = x1[b][:, sl]

            g = gemm[b, sl]                    # [EPC, 4]
            uniq = np.unique(g)                # <= 30000 rows
            tabk = np.zeros((NTAB, 256), np.float16)
            tabk[:len(uniq)] = mtab[b][uniq]
            m[f"tab{b}"] = tabk
            remap = np.searchsorted(uniq, g).astype(np.int64)  # [EPC, 4]
            for p in range(2):
                j = b * 2 + p
                vA = np.zeros(NP, np.int16)
                vA[:EPC] = remap[:, p]
                vB = np.zeros(NP, np.int16)
                vB[:EPC] = remap[:, p + 2]
                base = j * (2 * NP // 16)
                for c0, cw in zip(CHOFF, CHS):
                    blk = np.concatenate([vA[c0:c0 + cw], vB[c0:c0 + cw]])
                    off = base + 2 * c0 // 16
                    idx_host[:, off:off + 2 * cw // 16] = _wrap_idx(blk)
        m["xcm"] = xcm
        m["idxs"] = idx_host
        in_maps.append(m)
    return in_maps


def _assemble(results):
    out = np.zeros((B, 3, E), np.float32)
    for k in range(NCORES):
        sl = slice(k * EPC, (k + 1) * EPC)
        rD = results[k]["outD"]
        rP = results[k]["outP"]
        for b in range(B):
            out[b][:, sl] = (rD[b][:, :EPC]
                             + rP[b * 2 + 0][:, :EPC]
                             + rP[b * 2 + 1][:, :EPC])
    return out.reshape(B, 1, 3, E)


def kernel(**inputs):
    in_maps = _prep_inputs(inputs)
    if 1 not in _compiled:
        _compiled[1] = _build_program(1)
    nc = _compiled[1]
    res = run_bass_kernel_spmd(nc, in_maps, list(range(NCORES)))
    return _assemble(res.results)


if __name__ == "__main__":
    rng = np.random.default_rng(0)
    ins = {
        "x_0": rng.standard_normal((B, C, E)).astype(np.float32),
        "x_1": rng.standard_normal((B, C, E)).astype(np.float32),
        "gemm": rng.integers(0, E, (B, E, 4)).astype(np.int32),
        "Wa_local": (rng.standard_normal((C, C)) * 0.05).astype(np.float32),
        "ba_local": (rng.standard_normal(C) * 0.05).astype(np.float32),
        "Wb_local": (rng.standard_normal((C, C)) * 0.05).astype(np.float32),
        "bb_local": (rng.standard_normal(C) * 0.05).astype(np.float32),
        "Wa_tri": (rng.standard_normal((C, C, 5)) * 0.05).astype(np.float32),
        "ba_tri": (rng.standard_normal(C) * 0.05).astype(np.float32),
        "Wb_tri": (rng.standard_normal((C, C, 5)) * 0.05).astype(np.float32),
        "bb_tri": (rng.standard_normal(C) * 0.05).astype(np.float32),
        "Wa_fuse": (rng.standard_normal((3, 2 * C)) * 0.05).astype(np.float32),
        "ba_fuse": (rng.standard_normal(3) * 0.05).astype(np.float32),
        "Wb_fuse": (rng.standard_normal((3, 2 * C)) * 0.05).astype(np.float32),
        "bb_fuse": (rng.standard_normal(3) * 0.05).astype(np.float32),
    }
    y = kernel(**ins)

    def np_ref(i):
        o = np.zeros((B, 3, E), np.float32)
        for b in range(B):
            g = i["gemm"][b]
            for x, WL, bL, WT, bT, WF, bF in (
                (i["x_0"][b], i["Wa_local"], i["ba_local"], i["Wa_tri"],
                 i["ba_tri"], i["Wa_fuse"], i["ba_fuse"]),
                (i["x_1"][b], i["Wb_local"], i["bb_local"], i["Wb_tri"],
                 i["bb_tri"], i["Wb_fuse"], i["bb_fuse"]),
            ):
                loc = WL @ x + bL[:, None]
                f = x[:, g]  # [C, E, 4]
                G = np.stack([x, f[..., 0] + f[..., 2], f[..., 1] + f[..., 3],
                              np.abs(f[..., 0] - f[..., 2]),
                              np.abs(f[..., 1] - f[..., 3])], -1)
                tri = np.einsum("ces,ocs->oe", G, WT) + bT[:, None]
                o[b] += WF @ np.concatenate([loc, tri], 0) + bF[:, None]
        return o.reshape(B, 1, 3, E)

    exp = np_ref(ins)
    err = np.abs(y - exp).max() / np.abs(exp).max()
    print("max abs err:", np.abs(y - exp).max(), "rel:", err)


# revision 40
# speedup vs baseline: 558128.8202x; 1.1536x over previous
"""Trainium2 Bass kernel for the MeshCNN-style GNN message-passing block.

Math: the reference is
    out[b,:,e] = Wa_fuse @ [conv1(x0); mesh_conv(x0)] + Wb_fuse @ [conv1(x1); mesh_conv(x1)] + biases
Everything after the neighbor gather / abs is linear, so the whole block
collapses to ten [3,128] effective matrices applied to:
    x (direct), f1+f3, f2+f4, |f1-f3|, |f2-f4|      (for x0 and x1)
plus one bias 3-vector.  The kernel is gather-dominated (memory regime).

Strategy (8 cores, SPMD, edge-sharded):
- Each core owns E/8 = 7500 contiguous edges.  Its gathers reference at
  most 4*7500 = 30000 distinct rows, so a per-core COMPACTED table
  (dedup'd, host-remapped indices) always fits int16 addressing: one
  dma_gather table per batch, no lo/hi class split, no index classes.
- fp16 rows [x0[b];x1[b]] of 512B; dma_gather(transpose=True) lands
  neighbor features channel-major [128,2,N] for the PE matmul.
- |f1-f3| via DVE subtract + fused (*-1, max).
- The direct (conv1 + s=0) term streams the core's own edge slice
  channel-major (host pre-transposed) and accumulates via PE.
- Host sums outD + outP[p=0] + outP[p=1] per edge slice.
"""

import hashlib
import os
import shutil

import numpy as np

import concourse.bass as bass
import concourse.bacc as bacc
import concourse.tile as tile
from concourse import mybir
from concourse.bass_utils import run_bass_kernel_spmd

# ---- NEFF compile cache keyed on exact BIR bytes so repeat invocations of
# the same program skip neuronxcc. ----
_NEFF_CACHE = os.environ.get("KERNEL_NEFF_CACHE", "/tmp/neff_cache")
try:
    import concourse.bass2jax as _b2j

    if not hasattr(_b2j, "_orig_compile_bir_kernel"):
        _b2j._orig_compile_bir_kernel = _b2j.compile_bir_kernel

        def _cached_compile_bir_kernel(bir_json, tmpdir, neff_name="file.neff"):
            os.makedirs(_NEFF_CACHE, exist_ok=True)
            key = hashlib.sha256(bir_json).hexdigest()
            cpath = os.path.join(_NEFF_CACHE, key + ".neff")
            out = os.path.join(tmpdir, neff_name)
            if os.path.exists(cpath):
                shutil.copyfile(cpath, out)
                return out
            path = _b2j._orig_compile_bir_kernel(bir_json, tmpdir, neff_name)
            tmp = cpath + ".tmp"
            shutil.copyfile(path, tmp)
            os.replace(tmp, cpath)
            return path

        _b2j.compile_bir_kernel = _cached_compile_bir_kernel

    _b2j.install_neuronx_cc_hook()
    import libneuronxla as _lnx

    if hasattr(_lnx, "orig_neuronx_cc") and not hasattr(_lnx, "_ant_cc_cached"):
        _lnx._ant_cc_cached = True
        _orig_cc = _lnx.orig_neuronx_cc

        def _cached_cc(code, code_format, platform_version, file_prefix):
            os.makedirs(_NEFF_CACHE, exist_ok=True)
            key = hashlib.sha256(
                bytes(code) + bytes(code_format) + str(platform_version).encode()
            ).hexdigest()
            cpath = os.path.join(_NEFF_CACHE, key + ".cc")
            if os.path.exists(cpath):
                with open(cpath, "rb") as f:
                    return 0, f.read()
            r = _orig_cc(code, code_format, platform_version, file_prefix)
            try:
                rc, blob = r
                if rc == 0 and isinstance(blob, (bytes, bytearray)):
                    tmp = cpath + ".tmp"
                    with open(tmp, "wb") as f:
                        f.write(blob)
                    os.replace(tmp, cpath)
            except Exception:
                pass
            return r

        _lnx.orig_neuronx_cc = _cached_cc
except Exception:
    pass

B, C, E = 2, 128, 60000
NCORES = 8
EPC = E // NCORES              # 7500 edges per core
NP = 7552                      # padded edge count
CHS = (3840, 3712)             # gather chunk sizes (each a multiple of 128)
CHOFF = (0, 3840)              # chunk column offsets
NTAB = 30016                   # static compact-table rows (>= 4*7500)

F16 = mybir.dt.float16
F32 = mybir.dt.float32
I16 = mybir.dt.int16

_compiled = {}


def _mm_slices(n):
    """Split [0,n) into PSUM-bank-aligned matmul slices of <=512."""
    out = []
    a = 0
    while a < n:
        w = min(512, n - a)
        out.append((a, w))
        a += w
    return out


def _build_program(reps=1, nq=4, scratch=16384, qstride=2,
                   do_gather=True, do_trans=True, do_dve=True, do_mm=True):
    # Multi-queue SWDGE quadruples gather descriptor-generation throughput
    # (each queue runs on its own Q7 cpu pair), but the XBAR-transpose spray
    # path is not multi-queue safe (concurrent sprays corrupt data), so the
    # gathers land row-major (transpose=False) and the PE transposes blocks
    # into PSUM via is_transpose matmuls against an identity matrix.  DVE
    # then computes s = f1+f3 and |f1-f3| straight from PSUM into SBUF,
    # which also halves the downstream matmul count.
    nc = bacc.Bacc("TRN2", target_bir_lowering=False, debug=False,
                   num_devices=NCORES, num_swdge_queues=nq,
                   dynamic_dma_scratch_size=scratch)
    swdge_emit = [0]

    def next_q():
        q = swdge_emit[0] % nq
        swdge_emit[0] += 1
        return q

    tabs = [nc.dram_tensor(f"tab{b}", [NTAB, 256], F16, kind="ExternalInput")
            for b in range(B)]
    xcm_d = nc.dram_tensor("xcm", [B, 128, 2, NP], F16, kind="ExternalInput")
    idx_d = nc.dram_tensor("idxs", [128, NP // 2], I16, kind="ExternalInput")
    wts_d = nc.dram_tensor("wts", [128, 30], F16, kind="ExternalInput")
    bias_d = nc.dram_tensor("bias3", [3, 1], F32, kind="ExternalInput")
    idn_d = nc.dram_tensor("idn", [128, 128], F16, kind="ExternalInput")
    outP_d = nc.dram_tensor("outP", [4, 3, NP], F32, kind="ExternalOutput")
    outD_d = nc.dram_tensor("outD", [B, 3, NP], F32, kind="ExternalOutput")

    ACT_COPY = mybir.ActivationFunctionType.Copy
    ACT_IDENT = mybir.ActivationFunctionType.Identity
    SUB = mybir.AluOpType.subtract
    ADD = mybir.AluOpType.add

    with tile.TileContext(nc) as tc:
        with (
            tc.tile_pool(name="const", bufs=1) as cp,
            tc.tile_pool(name="sb", bufs=2) as sb,
            tc.tile_pool(name="ps", bufs=2, space="PSUM") as ps,
            tc.tile_pool(name="pst", bufs=3, space="PSUM") as pst,
        ):
            wts_t = cp.tile([128, 30], F16)
            nc.sync.dma_start(out=wts_t[:], in_=wts_d[:])
            bias_t = cp.tile([3, 1], F32)
            nc.sync.dma_start(out=bias_t[:], in_=bias_d[:])
            idx_t = cp.tile([128, NP // 2], I16)
            nc.sync.dma_start(out=idx_t[:], in_=idx_d[:])
            idn_t = cp.tile([128, 128], F16)
            nc.sync.dma_start(out=idn_t[:], in_=idn_d[:])

            for _rep in range(reps):
                for b in range(B):
                    # ---- direct term: A0 @ x0cm + B0 @ x1cm + bias ----
                    for c0, cw in zip(CHOFF, CHS):
                        xt = sb.tile([128, 2, cw], F16, tag="xt")
                        nc.sync.dma_start(out=xt[:],
                                          in_=xcm_d[b, :, :, c0:c0 + cw])
                        od = sb.tile([3, cw], F32, tag="od")
                        for (a, w) in _mm_slices(cw):
                            pt = ps.tile([3, 512], F32, tag="ps")
                            nc.tensor.matmul(pt[:, :w], lhsT=wts_t[:, 0:3],
                                             rhs=xt[:, 0, a:a + w],
                                             start=True, stop=False)
                            nc.tensor.matmul(pt[:, :w], lhsT=wts_t[:, 3:6],
                                             rhs=xt[:, 1, a:a + w],
                                             start=False, stop=True)
                            nc.scalar.activation(od[:, a:a + w], pt[:, :w],
                                                 ACT_IDENT, bias=bias_t[:, 0:1])
                        nc.sync.dma_start(out=outD_d[b, :, c0:c0 + cw],
                                          in_=od[:])

                    # ---- gather passes ----
                    for p in range(2):
                        j = b * 2 + p
                        cA = 6 + 12 * p      # lin lhsT col for x0-side
                        cB = 9 + 12 * p
                        cA2 = 12 + 12 * p    # abs lhsT col
                        cB2 = 15 + 12 * p
                        for ci, (c0, cw) in enumerate(zip(CHOFF, CHS)):
                            i0 = j * (2 * NP // 16) + 2 * c0 // 16
                            g13 = sb.tile([128, 2 * cw // 128, 256], F16,
                                          tag="g13", bufs=3)
                            nc.gpsimd.dma_gather(
                                g13[:], tabs[b][:],
                                idx_t[:, i0:i0 + 2 * cw // 16],
                                num_idxs=2 * cw, num_idxs_reg=2 * cw,
                                elem_size=256, transpose=False,
                                single_packet=False, queue_num=next_q())
                            g1 = g13[:, 0:cw // 128, :]
                            g3 = g13[:, cw // 128:2 * cw // 128, :]
                            og = sb.tile([3, cw], F32, tag="og")
                            for (a, w) in _mm_slices(cw):
                                g1t = pst.tile([128, 2, 512], F16, tag="pt1")
                                g3t = pst.tile([128, 2, 512], F16, tag="pt2")
                                if do_trans:
                                    for kb in range(w // 128):
                                        eb = (a + kb * 128) // 128
                                        for chh in range(2):
                                            nc.tensor.matmul(
                                                g1t[:, chh, kb * 128:kb * 128 + 128],
                                                lhsT=g1[:, eb, chh * 128:chh * 128 + 128],
                                                rhs=idn_t[:],
                                                start=True, stop=True,
                                                is_transpose=True)
                                            nc.tensor.matmul(
                                                g3t[:, chh, kb * 128:kb * 128 + 128],
                                                lhsT=g3[:, eb, chh * 128:chh * 128 + 128],
                                                rhs=idn_t[:],
                                                start=True, stop=True,
                                                is_transpose=True)
                                g1s = sb.tile([128, 2, 512], F16, tag="g1s", bufs=5)
                                s = sb.tile([128, 2, 512], F16, tag="s", bufs=5)
                                dd = sb.tile([128, 2, 512], F16, tag="dd", bufs=5)
                                if not do_dve and do_mm:
                                    nc.scalar.activation(s[0:3, 0, 0:4],
                                                         wts_t[0:3, 0:4],
                                                         ACT_COPY)
                                    nc.scalar.activation(dd[0:3, 0, 0:4],
                                                         wts_t[0:3, 0:4],
                                                         ACT_COPY)
                                if do_dve:
                                    nc.scalar.activation(g1s[:, :, :w],
                                                         g1t[:, :, :w],
                                                         ACT_COPY)
                                    nc.vector.tensor_tensor(
                                        s[:, :, :w], g1s[:, :, :w],
                                        g3t[:, :, :w], op=ADD)
                                    nc.vector.tensor_tensor(
                                        dd[:, :, :w], g1s[:, :, :w],
                                        g3t[:, :, :w], op=SUB)
                                    # |d| = max(d * -1, d)
                                    nc.vector.scalar_tensor_tensor(
                                        dd[:, :, :w], dd[:, :, :w], -1.0,
                                        dd[:, :, :w],
                                        op0=mybir.AluOpType.mult,
                                        op1=mybir.AluOpType.max)
                                if do_mm:
                                    pt = ps.tile([3, 512], F32, tag="ps")
                                    nc.tensor.matmul(pt[:, :w],
                                                     lhsT=wts_t[:, cA:cA + 3],
                                                     rhs=s[:, 0, :w],
                                                     start=True, stop=False)
                                    nc.tensor.matmul(pt[:, :w],
                                                     lhsT=wts_t[:, cB:cB + 3],
                                                     rhs=s[:, 1, :w],
                                                     start=False, stop=False)
                                    nc.tensor.matmul(pt[:, :w],
                                                     lhsT=wts_t[:, cA2:cA2 + 3],
                                                     rhs=dd[:, 0, :w],
                                                     start=False, stop=False)
                                    nc.tensor.matmul(pt[:, :w],
                                                     lhsT=wts_t[:, cB2:cB2 + 3],
                                                     rhs=dd[:, 1, :w],
                                                     start=False, stop=True)
                                    nc.scalar.activation(og[:, a:a + w],
                                                         pt[:, :w], ACT_COPY)
                            if not do_mm:
                                nc.scalar.activation(og[0:3, 0:4],
                                                     wts_t[0:3, 0:4], ACT_COPY)
                            nc.sync.dma_start(
                                out=outP_d[j, :, c0:c0 + cw],
                                in_=og[:])

    # Post-schedule queue assignment: Tile round-robins SWDGE completions
    # over 8 DMASW sems in SCHEDULED order and its cumulative thresholds
    # assume in-order completion per sem, which only holds when all gathers
    # sharing a sem run on one FIFO HW queue.  queue_num is mutable after
    # scheduling, so spread sems over the 4 queues by row-count load.
    sem_groups = {}
    for blk in nc.m.functions[0].blocks:
        for inst in blk.instructions:
            if isinstance(inst, mybir.InstDMAGatherAnt) and inst.sync_info:
                sem_groups.setdefault(inst.sync_info.on_update[0].id,
                                      []).append(inst)
    loads = [0, 0, 0, 0]
    for load, insts in sorted(
            ((sum(i.num_idxs for i in g), g) for g in sem_groups.values()),
            key=lambda t: -t[0]):
        q = min(range(4), key=lambda k: loads[k])
        loads[q] += load
        for i in insts:
            i.queue_num = q

    nc.compile()
    return nc


def _wrap_idx(vals):
    """[L] int16 -> wrapped [128, L//16] (i at [i%16, i//16], 8x repl)."""
    w = vals.reshape(-1, 16).T
    return np.tile(w, (8, 1))


def _prep_inputs(inputs):
    """Fold weights + build per-core compact tables, indices, shards."""
    x0 = np.asarray(inputs["x_0"], np.float32)
    x1 = np.asarray(inputs["x_1"], np.float32)
    gemm = np.asarray(inputs["gemm"]).astype(np.int64)

    Wa_local = np.asarray(inputs["Wa_local"], np.float32)
    ba_local = np.asarray(inputs["ba_local"], np.float32)
    Wb_local = np.asarray(inputs["Wb_local"], np.float32)
    bb_local = np.asarray(inputs["bb_local"], np.float32)
    Wa_tri = np.asarray(inputs["Wa_tri"], np.float32)
    ba_tri = np.asarray(inputs["ba_tri"], np.float32)
    Wb_tri = np.asarray(inputs["Wb_tri"], np.float32)
    bb_tri = np.asarray(inputs["bb_tri"], np.float32)
    Wa_fuse = np.asarray(inputs["Wa_fuse"], np.float32)
    ba_fuse = np.asarray(inputs["ba_fuse"], np.float32)
    Wb_fuse = np.asarray(inputs["Wb_fuse"], np.float32)
    bb_fuse = np.asarray(inputs["bb_fuse"], np.float32)

    # ---- fold weights to ten [3,128] effective matrices + bias ----
    Afl, Aft = Wa_fuse[:, :C], Wa_fuse[:, C:]
    Bfl, Bft = Wb_fuse[:, :C], Wb_fuse[:, C:]
    A0 = Afl @ Wa_local + Aft @ Wa_tri[:, :, 0]
    B0 = Bfl @ Wb_local + Bft @ Wb_tri[:, :, 0]
    A1, A2, A3, A4 = (Aft @ Wa_tri[:, :, s] for s in (1, 2, 3, 4))
    B1, B2, B3, B4 = (Bft @ Wb_tri[:, :, s] for s in (1, 2, 3, 4))
    bias = (ba_fuse + bb_fuse + Afl @ ba_local + Aft @ ba_tri
            + Bfl @ bb_local + Bft @ bb_tri)

    mats = [A0, B0, A1, B1, A3, B3, A2, B2, A4, B4]
    wts = np.zeros((128, 30), np.float16)
    for jm, M in enumerate(mats):
        wts[:, 3 * jm:3 * jm + 3] = M.T.astype(np.float16)

    # ---- master per-batch tables: row e = [x0[b,:,e]; x1[b,:,e]] fp16 ----
    mtab = np.empty((B, E, 256), np.float16)
    for b in range(B):
        mtab[b, :, :128] = x0[b].T
        mtab[b, :, 128:] = x1[b].T

    in_maps = []
    for k in range(NCORES):
        sl = slice(k * EPC, (k + 1) * EPC)
        m = {"wts": wts, "bias3": bias.astype(np.float32).reshape(3, 1),
             "idn": np.eye(128, dtype=np.float16)}

        xcm = np.zeros((B, 128, 2, NP), np.float16)
        idx_host = np.zeros((128, NP // 2), np.int16)
        for b in range(B):
            xcm[b, :, 0, :EPC] = x0[b][:, sl]
            xcm[b, :, 1, :EPC] = x1[b][:, sl]

            g = gemm[b, sl]                    # [EPC, 4]
            uniq = np.unique(g)                # <= 30000 rows
            tabk = np.zeros((NTAB, 256), np.float16)
            tabk[:len(uniq)] = mtab[b][uniq]
            m[f"tab{b}"] = tabk
            remap = np.searchsorted(uniq, g).astype(np.int64)  # [EPC, 4]
            for p in range(2):
                j = b * 2 + p
                vA = np.zeros(NP, np.int16)
                vA[:EPC] = remap[:, p]
                vB = np.zeros(NP, np.int16)
                vB[:EPC] = remap[:, p + 2]
                base = j * (2 * NP // 16)
                for c0, cw in zip(CHOFF, CHS):
                    blk = np.concatenate([vA[c0:c0 + cw], vB[c0:c0 + cw]])
                    off = base + 2 * c0 // 16
                    idx_host[:, off:off + 2 * cw // 16] = _wrap_idx(blk)
        m["xcm"] = xcm
        m["idxs"] = idx_host
        in_maps.append(m)
    return in_maps


def _assemble(results):
    out = np.zeros((B, 3, E), np.float32)
    for k in range(NCORES):
        sl = slice(k * EPC, (k + 1) * EPC)
        rD = results[k]["outD"]
        rP = results[k]["outP"]
        for b in range(B):
            out[b][:, sl] = (rD[b][:, :EPC]
                             + rP[b * 2 + 0][:, :EPC]
                             + rP[b * 2 + 1][:, :EPC])
    return out.reshape(B, 1, 3, E)


def kernel(**inputs):
    in_maps = _prep_inputs(inputs)
    if 1 not in _compiled:
        _compiled[1] = _build_program(1)
    nc = _compiled[1]
    res = run_bass_kernel_spmd(nc, in_maps, list(range(NCORES)))
    return _assemble(res.results)


if __name__ == "__main__":
    rng = np.random.default_rng(0)
    ins = {
        "x_0": rng.standard_normal((B, C, E)).astype(np.float32),
        "x_1": rng.standard_normal((B, C, E)).astype(np.float32),
        "gemm": rng.integers(0, E, (B, E, 4)).astype(np.int32),
        "Wa_local": (rng.standard_normal((C, C)) * 0.05).astype(np.float32),
        "ba_local": (rng.standard_normal(C) * 0.05).astype(np.float32),
        "Wb_local": (rng.standard_normal((C, C)) * 0.05).astype(np.float32),
        "bb_local": (rng.standard_normal(C) * 0.05).astype(np.float32),
        "Wa_tri": (rng.standard_normal((C, C, 5)) * 0.05).astype(np.float32),
        "ba_tri": (rng.standard_normal(C) * 0.05).astype(np.float32),
        "Wb_tri": (rng.standard_normal((C, C, 5)) * 0.05).astype(np.float32),
        "bb_tri": (rng.standard_normal(C) * 0.05).astype(np.float32),
        "Wa_fuse": (rng.standard_normal((3, 2 * C)) * 0.05).astype(np.float32),
        "ba_fuse": (rng.standard_normal(3) * 0.05).astype(np.float32),
        "Wb_fuse": (rng.standard_normal((3, 2 * C)) * 0.05).astype(np.float32),
        "bb_fuse": (rng.standard_normal(3) * 0.05).astype(np.float32),
    }
    y = kernel(**ins)

    def np_ref(i):
        o = np.zeros((B, 3, E), np.float32)
        for b in range(B):
            g = i["gemm"][b]
            for x, WL, bL, WT, bT, WF, bF in (
                (i["x_0"][b], i["Wa_local"], i["ba_local"], i["Wa_tri"],
                 i["ba_tri"], i["Wa_fuse"], i["ba_fuse"]),
                (i["x_1"][b], i["Wb_local"], i["bb_local"], i["Wb_tri"],
                 i["bb_tri"], i["Wb_fuse"], i["bb_fuse"]),
            ):
                loc = WL @ x + bL[:, None]
                f = x[:, g]  # [C, E, 4]
                G = np.stack([x, f[..., 0] + f[..., 2], f[..., 1] + f[..., 3],
                              np.abs(f[..., 0] - f[..., 2]),
                              np.abs(f[..., 1] - f[..., 3])], -1)
                tri = np.einsum("ces,ocs->oe", G, WT) + bT[:, None]
                o[b] += WF @ np.concatenate([loc, tri], 0) + bF[:, None]
        return o.reshape(B, 1, 3, E)

    exp = np_ref(ins)
    err = np.abs(y - exp).max() / np.abs(exp).max()
    print("max abs err:", np.abs(y - exp).max(), "rel:", err)
